# revision 52
# baseline (speedup 1.0000x reference)
"""Causal multi-head attention (ChunkedDotProdAttention) on 8 TRN2 NeuronCores.

Problem: q,k,v [2, 2048, 2048] f32, 16 heads of dh=128, causal mask
(masked scores set to -50000 -> softmax -> exactly 0 in f32), out = attn @ v.

Sharding: 32 (batch, head) pairs, 4 per core; each core computes full
attention for its pairs — no cross-device comm.

Per-core kernel layout trick: everything is computed transposed.
  - host pre-transposes q,k to [dh, n] (bf16) so no on-chip input transposes;
    v is host-packed to the exact SBUF layout for a flat contiguous DMA
  - S^T[k, q] = K_j^T.T @ Q^T per (key-block j, 1024-query chunk c), causal
    blocks only, narrowed to valid queries; one [128,1024] psum S tile per
    key block so the exp runs once per block (amortizes ACT access latency)
  - P^T = exp(scale * S^T): ACT is the bottleneck engine, so 5 of each
    head's 24 blocks compute exp on DVE instead via the Schraudolph
    bit-trick: bf16(x) ~ bitcast(int16(x*128/ln2 + 16250.5)) — one
    tensor_scalar(mult,add) with an int16-converting write (~2% rel err).
    Only LOW-WEIGHT-SHARE blocks are offloaded: block (c,0) is seen by
    every query of its chunk (~26% weight share for chunk 0), so sch'ing
    it doubles end-to-end error — keep (c,0) exact on ACT. Offloaded
    blocks use a dedicated 2-bank PSUM pool (ps_s2) + ~2-block-early
    emission so they run concurrently with the ACT chain's s rotation.
  - chunk/pair/body transitions software-pipelined: block 0's QK is
    pre-issued into ps_s2 during the previous chunk (PIPELINE_B0 pqk path;
    cross-body too), so the ACT exp chain never waits the s-rotation at a
    boundary.
  - each chunk's last 3 narrow blocks (384+256+128 cols) PACK into one
    s-tile (pieces bank-aligned: 384+128 in bank A, 256 in bank B — a
    matmul output must never cross a psum bank boundary!) with ONE merged
    ACT exp into a pt scratch region: 2 fewer ACT inits/chunk and the
    tail's QK->exp ping-pong collapses to a single step.
  - diagonal blocks: triangular zero via gpsimd affine_select (GPSIMD
    cannot touch PSUM, and affine_select compiles only on GPSIMD)
  - out^T[d, q] += V_j.T @ P^T_j accumulated in PSUM (V_j natural layout!)
  - softmax denominators: bf16 accumulators ALIASED into pt storage
    (acc_a = pt block 0, acc_b = pt block 1 — raw, no adds; acc2 = pt
    block jc-4 cols [512:]): the exp that writes those blocks IS the init,
    so there are no init copies. DVE runs all the adds (measured on HW:
    GPSIMD/Pool tensor_add costs ~1.6us/op on the critical path — the
    cost model badly underestimates it; POOL_ADDS stays False). The host
    sums the three exported partials over partitions and divides.
  - outputs stream out per 512-col half as their PSUM bank finalizes;
    next-pair input DMAs are prefetched before this pair's tail DMAs
  - bench loop (reps>0) unrolls 16 bodies per For_i iteration (back-edge
    barrier amortization; measured ~1us/body better than 8)
Engine busy (cost model): DVE ~59.7us, PE ~58.3us, ACT ~57.4us, Pool ~18us;
TimelineSim steady-state ~67.3us/body. Measured HW (same-session A/B):
this kernel ~80-86us/body vs baseline ~90-103us/body (10-16% faster);
1-core run ~72.6us/body => ~8us of the 8-core time is HBM/DMA contention.
HW rel err 6.65e-3.
"""

import numpy as np
import ml_dtypes

B = 2
N = 2048
D_MODEL = 2048
H = 16
DH = 128
N_CORES = 8
PAIRS_PER_CORE = (B * H) // N_CORES  # 4
SCALE = float(DH) ** -0.5
CHUNK = 1024  # query chunk (2 psum banks; one key-block row per S tile)
NCHUNKS = N // CHUNK  # 2
QB = 128  # query/key block
NB = N // QB  # 16 key blocks
BLOCKS_PER_CHUNK = CHUNK // QB  # 8
MMN = 512  # max matmul free dim (one psum bank of f32)

# (chunk, key-block) pairs whose exp runs on DVE via the Schraudolph
# int16/bf16 bit-trick instead of ACT (the bottleneck engine)
SCH_OFFLOAD = {(0, 2), (0, 4), (1, 4), (1, 7), (1, 10)}
PIPELINE_B0 = True
# pack each chunk's last 3 (narrow, 384+256+128-col) blocks into ONE s-tile
# + ONE merged ACT exp: 2 fewer ACT instr inits per chunk, and the tail's
# QK->exp rotation ping-pong (which can't hide sem latency on narrow
# blocks) collapses to a single step
PACK_TAIL = True
PACKW = 768  # 384+256+128 scratch cols appended to the pt tile
# HW-measured: Pool/GPSIMD tensor_add is ~3-5x slower than the cost model
# thinks (~1.6us/op on the critical path) — keep the adds on DVE
POOL_ADDS = False
# bench-loop unroll: the For_i back-edge carries an all-engine barrier +
# semaphore reset; running 2 bodies per iteration halves that cost and
# lets consecutive bodies pipeline through the tile-pool rotations
UNROLL = 16
import math as _math

SCH_A = SCALE * 128.0 / _math.log(2.0)
SCH_B = 16256.0 - 5.5

_nc_cache = {}
_last_in_maps = None


def _build_nc(reps=0, unroll=None, bodies=None):
    """Build + compile the per-core Bass kernel (same NEFF for all cores).

    reps>0 wraps the body in a dynamic For_i loop running it `reps` times —
    used only for wall-clock benchmarking (the work is idempotent)."""
    from contextlib import ExitStack

    import concourse.bass as bass
    import concourse.mybir as mybir
    import concourse.tile as tile
    from concourse import bacc
    from concourse import bass_isa

    dt_mm = mybir.dt.bfloat16
    f32 = mybir.dt.float32

    nc = bacc.Bacc(
        "TRN2",
        target_bir_lowering=False,
        debug=False,
        enable_asserts=False,
        num_devices=N_CORES,
    )
    P = PAIRS_PER_CORE
    qT_d = nc.dram_tensor("qT", [P, DH, N], dt_mm, kind="ExternalInput").ap()
    kT_d = nc.dram_tensor("kT", [P, DH, N], dt_mm, kind="ExternalInput").ap()
    # v pre-arranged on host to the SBUF layout: [pair, k_local, block*DH+d]
    v_d = nc.dram_tensor("v", [P, QB, NB * DH], dt_mm, kind="ExternalInput").ap()
    # outT exported bf16: halves write-back bytes; host divides in f32 anyway
    outT_d = nc.dram_tensor("outT", [P, DH, N], dt_mm, kind="ExternalOutput").ap()
    # raw softmax-denominator accumulators; the host does the partition-sum
    # and the division (kills the PAR->recip->mul tail chain on device)
    # two partial accumulators (acc_a = pt block 0, accumulated on DVE;
    # acc_b = pt block 1, exported raw) as one contiguous [QB, 2*CHUNK] slab
    accs_d = nc.dram_tensor(
        "accs", [P, NCHUNKS, QB, 2 * CHUNK], dt_mm, kind="ExternalOutput"
    ).ap()
    # separate accumulator for each chunk's last 4 (small, upper-col) blocks,
    # summed on GPSIMD so DVE's add backlog never trails the exp chain
    accs2_d = nc.dram_tensor(
        "accs2", [P, NCHUNKS, QB, MMN], dt_mm, kind="ExternalOutput"
    ).ap()

    with tile.TileContext(nc) as tc, ExitStack() as ctx:
        sb = ctx.enter_context(tc.tile_pool(name="sb", bufs=3))
        pt_pool = ctx.enter_context(tc.tile_pool(name="pt", bufs=3))
        small = ctx.enter_context(tc.tile_pool(name="small", bufs=3))
        outp = ctx.enter_context(tc.tile_pool(name="outp", bufs=2))
        const_pool = ctx.enter_context(tc.tile_pool(name="const", bufs=1))
        ps_s = ctx.enter_context(tc.tile_pool(name="ps_s", bufs=2, space="PSUM"))
        # separate PSUM pool for the DVE-offloaded (Schraudolph) blocks so
        # they stay out of the ACT exp chain's s-buffer rotation
        ps_s2 = ctx.enter_context(tc.tile_pool(name="ps_s2", bufs=1, space="PSUM"))
        # o bufs=1: next chunk's first PV waits for its own QK+exp anyway,
        # by which time the previous chunk's copies are long done
        ps_o = ctx.enter_context(tc.tile_pool(name="ps_o", bufs=1, space="PSUM"))

        if unroll is None:
            unroll = UNROLL
        unroll = unroll if reps >= unroll else 1
        if reps:
            assert reps % unroll == 0, (reps, unroll)
        rep_ctx = ExitStack()
        if reps:
            # hint_engines: body >256 instrs/engine -> back-edge would
            # IRAM-miss (~4us) without prefetch hints; keeps the bench
            # closer to true single-shot time
            rep_ctx.enter_context(
                tc.For_i(
                    0,
                    reps // unroll,
                    1,
                    hint_engines=(
                        mybir.EngineType.PE,
                        mybir.EngineType.Activation,
                        mybir.EngineType.DVE,
                        mybir.EngineType.Pool,
                        mybir.EngineType.SP,
                    ),
                )
            )

        def emit_exp_dve(st, c, j):
            # Schraudolph bf16 exp on DVE (ACT is the bottleneck engine):
            # pt = bitcast_bf16(int16(s*A + B)) ~= exp(s*SCALE) to ~2% rel,
            # one tensor_scalar (mult,add) with an int16-converting write.
            # QK goes to the dedicated ps_s2 pool and this cluster is emitted
            # ~2 blocks early, so it runs concurrently with the ACT exp chain
            # instead of serializing into its s-buffer rotation.
            qoff = max(0, j * QB - c * CHUNK)
            pieces = [(qoff, MMN), (MMN, CHUNK)] if qoff < MMN else [(qoff, CHUNK)]
            s_ps = ps_s2.tile([128, CHUNK], f32, tag="s2")
            # below the ACT-chain QKs (offset 24): DVE's exp has ~2 blocks
            # of slack, the ACT chain has none
            with tc.high_priority(offset=8):
                for p0, p1 in pieces:
                    nc.tensor.matmul(
                        s_ps[:, p0:p1],
                        lhsT=st["kT"][:, j * QB : (j + 1) * QB],
                        rhs=st["qT"][:, c * CHUNK + p0 : c * CHUNK + p1],
                        start=True,
                        stop=True,
                    )
            nc.vector.tensor_scalar(
                st["pt"][:, j * CHUNK + qoff : (j + 1) * CHUNK].bitcast(
                    mybir.dt.int16
                ),
                s_ps[:, qoff:],
                SCH_A,
                SCH_B,
                mybir.AluOpType.mult,
                mybir.AluOpType.add,
            )

        def emit_exp_act(st, c, j):
            # QK + ACT exp for one block (the ACT exp chain)
            qoff = max(0, j * QB - c * CHUNK)
            pieces = [(qoff, MMN), (MMN, CHUNK)] if qoff < MMN else [(qoff, CHUNK)]
            s_ps = ps_s.tile([128, CHUNK], f32, tag="s")
            # hoist QKs ahead of the previous block's exp-gated PV cluster
            # (and the previous chunk/pair tail) in the static schedule;
            # the first blocks of a chunk hoist hardest so their QKs
            # clear PE before the previous chunk's exp-gated PV cluster
            # head-of-line blocks them
            with tc.high_priority(offset=100 if j < 2 else 24):
                for p0, p1 in pieces:
                    nc.tensor.matmul(
                        s_ps[:, p0:p1],
                        lhsT=st["kT"][:, j * QB : (j + 1) * QB],
                        rhs=st["qT"][:, c * CHUNK + p0 : c * CHUNK + p1],
                        start=True,
                        stop=True,
                    )
            nc.scalar.activation(
                st["pt"][:, j * CHUNK + qoff : (j + 1) * CHUNK],
                s_ps[:, qoff:],
                mybir.ActivationFunctionType.Exp,
                scale=SCALE,
            )

        def emit_qk_s2(st, c, j):
            # QK for the next chunk's block 0, into the s2 pool so it can
            # run during the current chunk's tail without joining the ACT
            # chain's s-buffer rotation (which would deadlock/serialize)
            s_ps = ps_s2.tile([128, CHUNK], f32, tag="s2")
            with tc.high_priority(offset=100):
                for p0, p1 in ((0, MMN), (MMN, CHUNK)):
                    nc.tensor.matmul(
                        s_ps[:, p0:p1],
                        lhsT=st["kT"][:, j * QB : (j + 1) * QB],
                        rhs=st["qT"][:, c * CHUNK + p0 : c * CHUNK + p1],
                        start=True,
                        stop=True,
                    )
            return s_ps

        def emit_block(st, c, j, jc):
            qoff = max(0, j * QB - c * CHUNK)
            # split [qoff, CHUNK) into <=MMN psum-bank-aligned pieces
            pieces = [(qoff, MMN), (MMN, CHUNK)] if qoff < MMN else [(qoff, CHUNK)]
            pt = st["pt"]
            if j == 0 and "pqk" in st:
                # QK was pre-issued into s2 during the previous chunk; the
                # exp lands here at its natural ACT-FIFO position
                nc.scalar.activation(
                    pt[:, 0:CHUNK],
                    st["pqk"][:, :],
                    mybir.ActivationFunctionType.Exp,
                    scale=SCALE,
                )
            elif (c, j) in SCH_OFFLOAD and j == 0 and 0 not in st["pre"]:
                # body-first chunk (no cross-chunk pre-issue): the sch
                # cluster must be emitted BEFORE this block's PV reads pt0
                emit_exp_dve(st, c, 0)
                st["pre"].add(0)
            elif (c, j) not in SCH_OFFLOAD and j not in st["pre"]:
                emit_exp_act(st, c, j)
            if j * QB >= c * CHUNK:  # diagonal block
                # strict-upper triangle of the first 128 valid cols:
                # keep where local_q - local_k >= 0
                nc.gpsimd.affine_select(
                    out=pt[:, j * CHUNK + qoff : j * CHUNK + qoff + QB],
                    in_=pt[:, j * CHUNK + qoff : j * CHUNK + qoff + QB],
                    compare_op=mybir.AluOpType.is_ge,
                    fill=0.0,
                    base=0,
                    channel_multiplier=-1,
                    pattern=[[1, QB]],
                )
            for p0, p1 in pieces:
                nc.tensor.matmul(
                    st["o"][:, p0:p1],
                    lhsT=st["v"][:, j * DH : (j + 1) * DH],
                    rhs=pt[:, j * CHUNK + p0 : j * CHUNK + p1],
                    start=(j == 0),
                    stop=(j == jc - 1),
                )
            # running denominator accumulation (narrowed; spread across the
            # chunk instead of a tail-heavy tree fold). The accumulators are
            # ALIASED into pt storage: acc_a = pt block 0 (DVE chain),
            # acc_b = pt block 1 (Pool chain — GPSIMD can't touch PSUM, so
            # SBUF adds are the only way to give it real work), acc2 = pt
            # block (jc-4)'s cols [MMN, CHUNK) (DVE) — the exp for those
            # blocks IS the init, killing the init copies. Safe because each
            # block's PV (the only other pt reader) runs before the next
            # block's add mutates the aliased region. The host sums all
            # three partial accumulators. Two parallel chains keep both
            # engines busy without cross-gating.
            if j in (0, 1, jc - 4):
                pass  # exp already wrote the aliased accumulator
            elif j < jc - 4:
                # Pool takes only chunk1's early adds: its queue there is
                # AS-free (no diagonal blocks until j=8), so the slow Q7
                # adds never delay an affine_select that gates a PV
                if POOL_ADDS and c == 1 and j in (2, 3):
                    nc.gpsimd.tensor_add(
                        pt[:, CHUNK + qoff : 2 * CHUNK],
                        pt[:, CHUNK + qoff : 2 * CHUNK],
                        pt[:, j * CHUNK + qoff : (j + 1) * CHUNK],
                    )
                else:
                    nc.vector.tensor_add(
                        pt[:, qoff:CHUNK],
                        pt[:, qoff:CHUNK],
                        pt[:, j * CHUNK + qoff : (j + 1) * CHUNK],
                    )
            else:
                nc.vector.tensor_add(
                    pt[:, (jc - 4) * CHUNK + qoff : (jc - 3) * CHUNK],
                    pt[:, (jc - 4) * CHUNK + qoff : (jc - 3) * CHUNK],
                    pt[:, j * CHUNK + qoff : (j + 1) * CHUNK],
                )
            # emit upcoming offloaded blocks' QK+exp_dve clusters ~2 blocks
            # ahead of their consume position
            for co, jo in sorted(SCH_OFFLOAD):
                if co == c and max(0, jo - 2) == j and jo not in st["pre"]:
                    emit_exp_dve(st, c, jo)
            # blocks past 4+8c only touch the upper PSUM bank, and past this
            # point acc is final too (later blocks go to acc2): export the
            # lower outT half and the whole acc now, off the chunk tail
            if j == 4 + 8 * c - 1:
                nc.vector.tensor_copy(st["outT_c"][:, :MMN], st["o"][:, :MMN])
                nc.sync.dma_start(
                    outT_d[st["p"]][:, c * CHUNK : c * CHUNK + MMN],
                    st["outT_c"][:, :MMN],
                )
                nc.sync.dma_start(accs_d[st["p"], c], pt[:, 0 : 2 * CHUNK])

        def emit_packed_tail(st, c, jc):
            # last 3 (narrow, diagonal) blocks: QKs packed into ONE s tile,
            # ONE merged exp into the pt scratch region, then per-block
            # AS/PV/acc2-add reading the packed pieces
            pt = st["pt"]
            PACK = NB * CHUNK
            s_ps = ps_s.tile([128, CHUNK], f32, tag="s")
            # piece offsets chosen so no matmul output crosses a psum bank
            # boundary: widths 384+128 fill bank A exactly, 256 in bank B
            offs = [
                (jc - 3, (jc - 3) * QB - c * CHUNK, 384, 0),
                (jc - 2, (jc - 2) * QB - c * CHUNK, 256, 512),
                (jc - 1, (jc - 1) * QB - c * CHUNK, 128, 384),
            ]
            for j, qoff, w, o in offs:
                with tc.high_priority(offset=24):
                    nc.tensor.matmul(
                        s_ps[:, o : o + w],
                        lhsT=st["kT"][:, j * QB : (j + 1) * QB],
                        rhs=st["qT"][:, c * CHUNK + qoff : (c + 1) * CHUNK],
                        start=True,
                        stop=True,
                    )
            nc.scalar.activation(
                pt[:, PACK : PACK + 768],
                s_ps[:, :768],
                mybir.ActivationFunctionType.Exp,
                scale=SCALE,
            )
            for j, qoff, w, o in offs:
                nc.gpsimd.affine_select(
                    out=pt[:, PACK + o : PACK + o + QB],
                    in_=pt[:, PACK + o : PACK + o + QB],
                    compare_op=mybir.AluOpType.is_ge,
                    fill=0.0,
                    base=0,
                    channel_multiplier=-1,
                    pattern=[[1, QB]],
                )
                nc.tensor.matmul(
                    st["o"][:, qoff:CHUNK],
                    lhsT=st["v"][:, j * DH : (j + 1) * DH],
                    rhs=pt[:, PACK + o : PACK + o + w],
                    start=False,
                    stop=(j == jc - 1),
                )
                nc.vector.tensor_add(
                    pt[:, (jc - 4) * CHUNK + qoff : (jc - 3) * CHUNK],
                    pt[:, (jc - 4) * CHUNK + qoff : (jc - 3) * CHUNK],
                    pt[:, PACK + o : PACK + o + w],
                )

        def emit_tail(st, c, last=False):
            # unnormalized out^T to SBUF; acc straight out to DRAM — the
            # host finishes the softmax division. The lower halves went out
            # mid-chunk (see emit_block); only the upper halves remain here.
            # For the very last chunk, ACT (idle after its final exp) does
            # the copy and fires the accs DMA from its own HWDGE queue so
            # the two tail DMAs overlap.
            jc = BLOCKS_PER_CHUNK * (c + 1)
            acc2 = st["pt"][:, (jc - 4) * CHUNK + MMN : (jc - 3) * CHUNK]
            if last:
                nc.scalar.copy(st["outT_c"][:, MMN:], st["o"][:, MMN:])
                nc.scalar.dma_start(accs2_d[st["p"], c], acc2)
            else:
                nc.vector.tensor_copy(st["outT_c"][:, MMN:], st["o"][:, MMN:])
                nc.sync.dma_start(accs2_d[st["p"], c], acc2)
            nc.sync.dma_start(
                outT_d[st["p"]][:, c * CHUNK + MMN : (c + 1) * CHUNK],
                st["outT_c"][:, MMN:],
            )

        def emit_pair_dmas(p):
            # DMA order = first-use order, finest pieces first so the first
            # QK (kT block 0, qT cols 0:512) unblocks as early as possible
            qT_s = sb.tile([128, N], dt_mm, tag="qT", name="qT_s")
            kT_s = sb.tile([128, N], dt_mm, tag="kT", name="kT_s")
            v_s = sb.tile([128, NB * DH], dt_mm, tag="v", name="v_s")
            nc.sync.dma_start(kT_s[:, : 2 * QB], kT_d[p][:, : 2 * QB])
            nc.sync.dma_start(qT_s[:, :MMN], qT_d[p][:, :MMN])
            nc.sync.dma_start(qT_s[:, MMN:CHUNK], qT_d[p][:, MMN:CHUNK])
            nc.sync.dma_start(v_s[:, : 4 * DH], v_d[p][:, : 4 * DH])
            nc.sync.dma_start(kT_s[:, 2 * QB :], kT_d[p][:, 2 * QB :])
            nc.sync.dma_start(v_s[:, 4 * DH :], v_d[p][:, 4 * DH :])
            nc.sync.dma_start(qT_s[:, CHUNK:], qT_d[p][:, CHUNK:])
            return {"qT": qT_s, "kT": kT_s, "v": v_s}

        def new_st(p, pair_t):
            st = {"p": p, "pre": set(), **pair_t}
            # +PACKW cols: scratch region for the packed-tail exp output
            st["pt"] = pt_pool.tile(
                [128, NB * CHUNK + PACKW], dt_mm, tag="pt", name="pt"
            )
            st["o"] = ps_o.tile([128, CHUNK], f32, tag="o", name="o")
            st["outT_c"] = outp.tile([128, CHUNK], dt_mm, tag="outT", name="outT_c")
            return st

        def emit_body(first_tiles=None, first_st=None, prefetch_next_body=False):
            pair_tiles = {0: first_tiles or emit_pair_dmas(0)}
            sts = {}
            if first_st is not None:
                sts[(0, 0)] = first_st
            return emit_body_inner(pair_tiles, sts, prefetch_next_body)

        def emit_body_inner(pair_tiles, sts, prefetch_next_body):
            def get_st(p, c):
                if (p, c) not in sts:
                    sts[(p, c)] = new_st(p, pair_tiles[p])
                return sts[(p, c)]

            seq = [(p, c) for p in range(P) for c in range(NCHUNKS)]
            next_body_tiles = None
            next_body_st = None
            for idx, (p, c) in enumerate(seq):
                st = get_st(p, c)
                if c == 1 and p + 1 < P:
                    # prefetch the next pair's inputs now, so their DMAs
                    # enqueue ahead of this chunk's tail DMAs in SP order
                    # (each DMA carries ~1.8us pre-transfer latency)
                    pair_tiles[p + 1] = emit_pair_dmas(p + 1)
                elif c == 1 and prefetch_next_body:
                    # same trick across the unrolled-body boundary: the next
                    # body's pair-0 inputs prefetch ahead of this body's tail
                    next_body_tiles = emit_pair_dmas(0)
                jc = BLOCKS_PER_CHUNK * (c + 1)  # key blocks 0..jc-1
                jtop = jc - 3 if PACK_TAIL else jc
                pipe_at = jc - 4 if PACK_TAIL else jc - 3
                for j in range(jtop):
                    emit_block(st, c, j, jc)
                    if PIPELINE_B0 and j == pipe_at:
                        # software-pipeline the chunk transition: pre-issue the
                        # next chunk's block-0 QK here (exp stays at its natural
                        # position) so the ACT chain never bubbles across
                        # chunk/pair/body boundaries
                        if idx + 1 < len(seq):
                            np_, nc_ = seq[idx + 1]
                            nst = get_st(np_, nc_)
                        elif next_body_tiles is not None:
                            nst = next_body_st = new_st(0, next_body_tiles)
                            nc_ = 0
                        else:
                            nst = None
                        if nst is not None:
                            if (nc_, 0) in SCH_OFFLOAD:
                                emit_exp_dve(nst, nc_, 0)
                                nst["pre"].add(0)
                            else:
                                nst["pqk"] = emit_qk_s2(nst, nc_, 0)
                if PACK_TAIL:
                    emit_packed_tail(st, c, jc)
                emit_tail(st, c, last=(p == P - 1 and c == NCHUNKS - 1))
            return next_body_tiles, next_body_st

        nxt = None
        nxt_st = None
        n_bodies = bodies if bodies else (unroll if reps else 1)
        for _rep in range(n_bodies):
            nxt, nxt_st = emit_body(
                first_tiles=nxt,
                first_st=nxt_st,
                prefetch_next_body=(_rep + 1 < n_bodies),
            )

        rep_ctx.close()

    nc.compile()
    return nc


def _get_nc():
    if "nc" not in _nc_cache:
        _nc_cache["nc"] = _build_nc()
    return _nc_cache["nc"]


def kernel(q, k, v):
    from concourse.bass_utils import run_bass_kernel_spmd

    q = np.asarray(q, dtype=np.float32)
    k = np.asarray(k, dtype=np.float32)
    v = np.asarray(v, dtype=np.float32)

    bf16 = ml_dtypes.bfloat16
    # [b, n, h, dh] -> [b, h, dh, n] for q/k; [b, h, n, dh] for v
    qT = np.ascontiguousarray(
        q.reshape(B, N, H, DH).transpose(0, 2, 3, 1)
    ).astype(bf16)
    kT = np.ascontiguousarray(
        k.reshape(B, N, H, DH).transpose(0, 2, 3, 1)
    ).astype(bf16)
    # v -> [b, h, k_local, block, dh]: v_host[p, k, j*DH+d] = v[p, j*QB+k, d]
    vh = np.ascontiguousarray(
        v.reshape(B, NB, QB, H, DH).transpose(0, 3, 2, 1, 4)
    ).astype(bf16)

    qT = qT.reshape(B * H, DH, N)
    kT = kT.reshape(B * H, DH, N)
    vh = vh.reshape(B * H, QB, NB * DH)

    in_maps = []
    for core in range(N_CORES):
        lo = core * PAIRS_PER_CORE
        hi = lo + PAIRS_PER_CORE
        in_maps.append(
            {
                "qT": np.ascontiguousarray(qT[lo:hi]),
                "kT": np.ascontiguousarray(kT[lo:hi]),
                "v": np.ascontiguousarray(vh[lo:hi]),
            }
        )

    global _last_in_maps
    _last_in_maps = in_maps

    nc = _get_nc()
    res = run_bass_kernel_spmd(nc, in_maps, core_ids=list(range(N_CORES)))

    # reassemble: outT per core [P, dh, n] f32 (unnormalized) -> normalize by
    # the softmax denominators (sum the raw accumulators over partitions),
    # then back to [b, n, h*dh]
    outT = np.concatenate([r["outT"] for r in res.results], axis=0).astype(
        np.float32
    )  # [32, dh, n]
    accs = np.concatenate([r["accs"] for r in res.results], axis=0)
    accs2 = np.concatenate([r["accs2"] for r in res.results], axis=0)
    # accs: [32, NCHUNKS, 128, 2*CHUNK] bf16 — acc_a (cols :CHUNK) + acc_b
    # (cols CHUNK:); acc_b for chunk 0 aliases key-block 1, whose first 128
    # cols were never written (fully masked) -> exclude. accs2 holds the
    # last-4-blocks' partials for each chunk's upper cols [MMN, CHUNK)
    accs = accs.astype(np.float32)
    accs[:, 0, :, CHUNK : CHUNK + QB] = 0.0
    sums_c = (
        accs[:, :, :, :CHUNK].sum(axis=2) + accs[:, :, :, CHUNK:].sum(axis=2)
    )  # [32, NCHUNKS, CHUNK]
    sums_c[:, :, MMN:] += accs2.astype(np.float32).sum(axis=2)
    sums = sums_c.reshape(B * H, N)
    outT = outT / sums[:, None, :]
    out = outT.reshape(B, H, DH, N).transpose(0, 3, 1, 2).reshape(B, N, D_MODEL)
    return np.ascontiguousarray(out)

